# revision 1
# baseline (speedup 1.0000x reference)
"""Multi-head causal attention (B=4, S=4096, E=512, H=8) on 8 trn2 NeuronCores.

Sharding: core = (batch b, head-group g of 4 heads); 4 batches x 2 groups = 8 cores.
Each core computes qkv projection for its group's heads, causal attention, and a
partial output projection (its heads' rows of Wo). Host sums the two partials per
batch and adds bo.

Device layout (per core):
  xT   [512, 4096]   x[b] transposed (host-side) -> contraction dim on partitions
  qT/kT stored [128(2 heads' dh), 1024-token tiles]  (qkvT = W.T @ x.T on PE)
  V    stored token-major [128, kb*260 + h*65 + d] with a ones column per
       (kb, head) at d=64 -> the PV matmul lhsT [Vh|1] yields attention output
       in [dh, tok] layout AND softmax denominators in one pass.
  S_T  [128 keys, 1024 queries] in PSUM per 128-key block, causal-trapezoid
       column ranges; exp on ACT (scale=1/8 folded in); PV accumulates over
       key blocks in PSUM.
"""

import sys

sys.path.insert(0, "/opt/trn_rl_repo")

import numpy as np

B, S, E = 4, 4096, 512
H = 8
DH = 64
HPG = 4  # heads per group
GQ = 256  # features per group for each of q/k/v (HPG*DH)
QE = 1024  # query extent per attention sweep
NQQ = S // QE  # 4
NKB = S // 128  # 32
NTQ = 4  # token chunks for projection phase
TQ = S // NTQ  # 1024
VW = HPG * 65  # 260: per-key-block V width incl. ones columns
NEG = -1.0e10
SCALE = 0.125  # 1/sqrt(DH)

_CACHE = {}


def _chunks(qs, hi):
    """Split [qs, hi) into pieces that never cross a 512-column PSUM bank
    boundary (one matmul output must stay within a single PSUM bank)."""
    out = []
    for c0 in range(0, hi, 512):
        j0, j1 = max(qs, c0), min(hi, c0 + 512)
        if j0 < j1:
            out.append((j0, j1))
    return out



def _build_nc(repeat=1):
    import concourse.bass as bass
    import concourse.tile as tile
    import concourse.mybir as mybir
    from concourse import bacc

    f32 = mybir.dt.float32
    f32r = mybir.dt.float32r
    AF = mybir.ActivationFunctionType
    ALU = mybir.AluOpType

    nc = bacc.Bacc("TRN2", target_bir_lowering=False, debug=False)

    xT = nc.dram_tensor("xT", [E, S], f32r, kind="ExternalInput").ap()
    wqk = nc.dram_tensor("wqk", [E, 512], f32r, kind="ExternalInput").ap()
    bqk = nc.dram_tensor("bqk", [128, 4], f32, kind="ExternalInput").ap()
    wv = nc.dram_tensor("wv", [E, GQ], f32r, kind="ExternalInput").ap()
    bv = nc.dram_tensor("bv", [1, GQ], f32r, kind="ExternalInput").ap()
    wo = nc.dram_tensor("wo", [DH, HPG * 512], f32r, kind="ExternalInput").ap()
    out = nc.dram_tensor("out", [S, E], f32, kind="ExternalOutput").ap()

    with tile.TileContext(nc) as tc:
        with (
            tc.tile_pool(name="consts", bufs=1) as cpool,
            tc.tile_pool(name="xt", bufs=4) as xtpool,
            tc.tile_pool(name="qkv", bufs=1) as qkvpool,
            tc.tile_pool(name="pt", bufs=3) as ptpool,
            tc.tile_pool(name="att", bufs=1) as attpool,
            tc.tile_pool(name="eps", bufs=2) as epool,
            tc.tile_pool(name="outs", bufs=1) as opool,
            # PSUM: 8 banks fully owned by the paired attention loops;
            # projection/Wo psum tiles share the same slots via tags.
            tc.tile_pool(name="st", bufs=1, space="PSUM") as stpool,
            tc.tile_pool(name="ov", bufs=1, space="PSUM") as ovpool,
        ):
            # ---- constants ----
            wqk_sb = cpool.tile([128, 4 * 512], f32r, name="wqk_sb")
            for ec in range(4):
                nc.sync.dma_start(
                    wqk_sb[:, ec * 512 : (ec + 1) * 512],
                    wqk[ec * 128 : (ec + 1) * 128, :],
                )
            wv_sb = cpool.tile([128, 4 * GQ], f32r, name="wv_sb")
            for ec in range(4):
                nc.sync.dma_start(
                    wv_sb[:, ec * GQ : (ec + 1) * GQ],
                    wv[ec * 128 : (ec + 1) * 128, :],
                )
            wo_sb = cpool.tile([DH, HPG * 512], f32r, name="wo_sb")
            nc.sync.dma_start(wo_sb[:], wo[:])
            bqk_sb = cpool.tile([128, 4], f32, name="bqk_sb")
            nc.sync.dma_start(bqk_sb[:], bqk[:])
            bv_sb = cpool.tile([1, GQ], f32r, name="bv_sb")
            nc.sync.dma_start(bv_sb[:], bv[:])
            onesf = cpool.tile([128, 128], f32, name="onesf")
            nc.vector.memset(onesf[:], 1.0)
            ones_row = cpool.tile([1, 128], f32r, name="ones_row")
            nc.vector.tensor_copy(ones_row[:], onesf[0:1, :])
            bf16 = mybir.dt.bfloat16
            maskf = cpool.tile([128, 128], f32, name="maskf")
            nc.vector.memset(maskf[:], 0.0)
            nc.gpsimd.affine_select(
                out=maskf[:], in_=maskf[:], compare_op=ALU.is_ge, fill=NEG,
                base=0, pattern=[[1, 128]], channel_multiplier=-1,
            )
            maskT = cpool.tile([128, 128], bf16, name="maskT")
            nc.vector.tensor_copy(maskT[:], maskf[:])
            identf = cpool.tile([128, 128], f32, name="identf")
            nc.vector.memset(identf[:], 0.0)
            nc.gpsimd.affine_select(
                out=identf[:], in_=identf[:], compare_op=ALU.not_equal, fill=1.0,
                base=0, pattern=[[-1, 128]], channel_multiplier=1,
            )
            ident = cpool.tile([128, 128], bf16, name="ident")
            nc.vector.tensor_copy(ident[:], identf[:])

            # persistent qT/kT tiles: [pair A/B][tq] each [128, 1024]
            # pair A rows 0:64 = head0 dh, 64:128 = head1; pair B = heads 2,3
            qt = [
                [qkvpool.tile([128, TQ], f32r, name=f"qt{ab}_{t}") for t in range(NTQ)]
                for ab in range(2)
            ]
            kt = [
                [qkvpool.tile([128, TQ], f32r, name=f"kt{ab}_{t}") for t in range(NTQ)]
                for ab in range(2)
            ]
            vt = [
                qkvpool.tile([128, 8 * VW], f32r, name=f"vt_{t}") for t in range(NTQ)
            ]

            def p1(tq):
                xts = []
                for ec in range(4):
                    xtile = xtpool.tile([128, TQ], f32r, name="xtile", tag="xtile")
                    nc.sync.dma_start(
                        xtile[:],
                        xT[ec * 128 : (ec + 1) * 128, tq * TQ : (tq + 1) * TQ],
                    )
                    xts.append(xtile)
                for gi, fc in enumerate((0, 2, 1, 3)):
                    dest = (qt if fc < 2 else kt)[fc % 2][tq]
                    for th in range(2):
                        tag = ("st_e", "st_o")[(gi * 2 + th) % 2]
                        ps = stpool.tile([128, 512], f32, name="mmps", tag=tag)
                        for ec in range(4):
                            nc.tensor.matmul(
                                ps[:],
                                lhsT=wqk_sb[:, ec * 512 + fc * 128 : ec * 512 + (fc + 1) * 128],
                                rhs=xts[ec][:, th * 512 : (th + 1) * 512],
                                start=(ec == 0),
                                stop=(ec == 3),
                            )
                        nc.vector.tensor_scalar_add(
                            dest[:, th * 512 : (th + 1) * 512],
                            ps[:],
                            bqk_sb[:, fc : fc + 1],
                        )
                v_tile = vt[tq]
                nc.vector.tensor_copy(
                    v_tile.rearrange("p (t h d) -> p t h d", t=8, h=HPG)[:, :, :, 64:65],
                    onesf[:, 0:32].rearrange("p (t h d) -> p t h d", t=8, h=HPG),
                )
                for tb in range(8):
                    vps = ovpool.tile([128, GQ], f32, name="vps", tag=("ov_e", "ov_o")[tb % 2])
                    for ec in range(4):
                        nc.tensor.matmul(
                            vps[:],
                            lhsT=xts[ec][:, tb * 128 : (tb + 1) * 128],
                            rhs=wv_sb[:, ec * GQ : (ec + 1) * GQ],
                            start=(ec == 0),
                            stop=False,
                        )
                    nc.tensor.matmul(
                        vps[:], lhsT=ones_row[:], rhs=bv_sb[:], start=False, stop=True
                    )
                    nc.vector.tensor_copy(
                        v_tile[:, tb * VW : (tb + 1) * VW].rearrange(
                            "p (h d) -> p h d", h=HPG
                        )[:, :, 0:64],
                        vps.rearrange("p (h d) -> p h d", h=HPG),
                    )

            atts = {}

            def epilogue(oc, ovt):
                # single copy releases the PSUM accumulator; row 64 = sums
                nc.vector.tensor_copy(oc[:], ovt[:])
                sbc = epool.tile([DH, QE], f32, name="sbc", tag="sbc")
                nc.sync.dma_start(
                    sbc[:],
                    oc[64:65, :].bitcast(f32).unsqueeze(1).to_broadcast([1, DH, QE]),
                )
                rbc = epool.tile([DH, QE], f32, name="rbc", tag="rbc")
                scr = epool.tile([DH, QE], f32, name="scr", tag="rscr", bufs=1)
                nc.vector.reciprocal_approx_accurate(out=rbc[:], in_=sbc[:], scratch=scr[:])
                nc.vector.tensor_tensor(oc[0:64, :], oc[0:64, :], rbc[:], ALU.mult)

            def att(qq, mid=None):
                atts[qq] = [
                    attpool.tile([65, QE], f32r, name=f"att_h{h}", tag=f"att{h}")
                    for h in range(HPG)
                ]
                nkb = 8 * qq + 8
                for pr in range(2):  # head pair (2pr, 2pr+1)
                    if pr == 1 and mid is not None:
                        mid()
                    ov_e = ovpool.tile([65, QE], f32, name="ov_e", tag="ov_e")
                    ov_o = ovpool.tile([65, QE], f32, name="ov_o", tag="ov_o")
                    for kb in range(nkb):
                        tqk, kbl = kb // 8, kb % 8
                        qs = max(0, (kb - 8 * qq) * 128)
                        st_e = stpool.tile([128, QE], f32, name="st_e", tag="st_e")
                        st_o = stpool.tile([128, QE], f32, name="st_o", tag="st_o")
                        for j0, j1 in _chunks(qs, QE):
                            # two concurrent row-tiled matmuls (rows 0:64 / 64:128)
                            nc.tensor.matmul(
                                st_e[:, j0:j1],
                                lhsT=kt[pr][tqk][0:64, kbl * 128 : (kbl + 1) * 128],
                                rhs=qt[pr][qq][0:64, j0:j1],
                                start=True,
                                stop=True,
                            )
                            nc.tensor.matmul(
                                st_o[:, j0:j1],
                                lhsT=kt[pr][tqk][64:128, kbl * 128 : (kbl + 1) * 128],
                                rhs=qt[pr][qq][64:128, j0:j1],
                                start=True,
                                stop=True,
                            )
                        if kb >= 8 * qq:  # diagonal: accumulate causal mask on PE
                            for stx in (st_e, st_o):
                                nc.tensor.matmul(
                                    stx[:, qs : qs + 128],
                                    lhsT=ident[:],
                                    rhs=maskT[:],
                                    start=False,
                                    stop=True,
                                    skip_group_check=True,
                                )
                        pt_e = ptpool.tile([128, QE], f32r, name="pt_e", tag="pt")
                        pt_o = ptpool.tile([128, QE], f32r, name="pt_o", tag="pt")
                        nc.scalar.activation(
                            pt_e[:, qs:QE], st_e[:, qs:QE], AF.Exp, bias=0.0, scale=SCALE
                        )
                        nc.scalar.activation(
                            pt_o[:, qs:QE], st_o[:, qs:QE], AF.Exp, bias=0.0, scale=SCALE
                        )
                        for j0, j1 in _chunks(qs, QE):
                            nc.tensor.matmul(
                                ov_e[:, j0:j1],
                                lhsT=vt[tqk][:, kbl * VW + 2 * pr * 65 : kbl * VW + (2 * pr + 1) * 65],
                                rhs=pt_e[:, j0:j1],
                                start=(kb == 0),
                                stop=(kb == nkb - 1),
                                skip_group_check=True,
                            )
                            nc.tensor.matmul(
                                ov_o[:, j0:j1],
                                lhsT=vt[tqk][:, kbl * VW + (2 * pr + 1) * 65 : kbl * VW + (2 * pr + 2) * 65],
                                rhs=pt_o[:, j0:j1],
                                start=(kb == 0),
                                stop=(kb == nkb - 1),
                                skip_group_check=True,
                            )
                    epilogue(atts[qq][2 * pr], ov_e)
                    epilogue(atts[qq][2 * pr + 1], ov_o)

            def wo(qq):
                att_h = atts[qq]
                out_sb = opool.tile([128, 4 * 512], f32, name="out_sb", tag="osb")
                for half in range(2):
                    for tb4 in range(4):
                        tb = half * 4 + tb4
                        wops = stpool.tile(
                            [128, 512], f32, name="wops", tag=("st_e", "st_o")[tb4 % 2]
                        )
                        for h in range(HPG):
                            nc.tensor.matmul(
                                wops[:],
                                lhsT=att_h[h][0:64, tb * 128 : (tb + 1) * 128],
                                rhs=wo_sb[:, h * 512 : (h + 1) * 512],
                                start=(h == 0),
                                stop=(h == HPG - 1),
                            )
                        nc.vector.tensor_copy(
                            out_sb[:, tb4 * 512 : (tb4 + 1) * 512], wops[:]
                        )
                    nc.sync.dma_start(
                        out[
                            qq * QE + half * 512 : qq * QE + (half + 1) * 512, :
                        ].rearrange("(t p) c -> p t c", p=128),
                        out_sb.rearrange("p (t c) -> p t c", t=4),
                    )

            def body(_i=None):
                for tq in range(NTQ):
                    p1(tq)
                att(0)
                att(1, mid=lambda: wo(0))
                att(2, mid=lambda: wo(1))
                att(3, mid=lambda: wo(2))
                wo(3)

            if repeat == 1:
                body()
            else:
                with tc.For_i(0, repeat, 1) as _i:
                    body(_i)

    nc.finalize()
    return nc


def _get_nc(repeat=1):
    key = ("nc", repeat)
    if key not in _CACHE:
        _CACHE[key] = _build_nc(repeat)
    return _CACHE[key]


def _make_in_maps(x, Wqkv, bqkv, Wo):
    in_maps = []
    for core in range(8):
        b, g = core // 2, core % 2
        qs, ks, vs = g * GQ, 512 + g * GQ, 1024 + g * GQ
        wqk_np = np.ascontiguousarray(
            np.concatenate([Wqkv[:, qs : qs + GQ], Wqkv[:, ks : ks + GQ]], axis=1)
        )
        bqk_np = np.ascontiguousarray(
            np.concatenate([bqkv[qs : qs + GQ], bqkv[ks : ks + GQ]]).reshape(4, 128).T
        )
        wv_np = np.ascontiguousarray(Wqkv[:, vs : vs + GQ])
        bv_np = np.ascontiguousarray(bqkv[vs : vs + GQ].reshape(1, GQ))
        wo_g = Wo[g * GQ : (g + 1) * GQ, :]
        wo_np = np.ascontiguousarray(
            np.concatenate([wo_g[h * DH : (h + 1) * DH, :] for h in range(HPG)], axis=1)
        )
        in_maps.append(
            {
                "xT": np.ascontiguousarray(x[b].T),
                "wqk": wqk_np,
                "bqk": bqk_np,
                "wv": wv_np,
                "bv": bv_np,
                "wo": wo_np,
            }
        )
    return in_maps


def kernel(x, Wqkv, bqkv, Wo, bo, **run_kwargs):
    from concourse.bass_utils import run_bass_kernel_spmd

    x = np.asarray(x, dtype=np.float32)
    Wqkv = np.asarray(Wqkv, dtype=np.float32)
    bqkv = np.asarray(bqkv, dtype=np.float32)
    Wo = np.asarray(Wo, dtype=np.float32)
    bo = np.asarray(bo, dtype=np.float32)

    nc = _get_nc()
    in_maps = _make_in_maps(x, Wqkv, bqkv, Wo)

    res = run_bass_kernel_spmd(nc, in_maps, core_ids=list(range(8)), **run_kwargs)
    _CACHE["last_results"] = res

    out = np.empty((B, S, E), dtype=np.float32)
    for b in range(B):
        out[b] = res.results[2 * b]["out"] + res.results[2 * b + 1]["out"] + bo
    return out



# revision 4
# speedup vs baseline: 1.4685x; 1.4685x over previous
"""Multi-head causal attention (B=4, S=4096, E=512, H=8) on 8 trn2 NeuronCores.

Sharding: core = (batch b, head-group g of 4 heads); 4 batches x 2 groups = 8 cores.
Each core computes qkv projection for its group's heads, causal attention, and a
partial output projection (its heads' rows of Wo). Host sums the two partials per
batch and adds bo.

All matmul operands are bf16 (fp32/f32r runs the PE in multi-pass mode; bf16
streams 1 col/cycle). PSUM accumulation stays fp32.

The attention loop is software-pipelined to keep the PE continuously busy (the
PE clock drops to half speed after any idle gap, which was the dominant cost):
  - 512-query sweeps; S^T for both heads of a pair lives in ONE [128, 1024]
    PSUM tile (head e cols 0:512, head o 512:1024), double-buffered (4 banks),
    so QK(kb+1) never waits for exp(kb).
  - PV runs one kb behind QK; exp(kb) overlaps QK(kb+1)+PV(kb-1).
  - ov accumulators [65, 512] x2 (2 banks); remaining 2 banks feed interleaved
    projection/Wo "filler" matmuls pulled one group per kb slot.
  - V is stored token-major with a ones column per (kb, head) so the PV matmul
    also produces softmax denominators (row 64 of ov).
"""

import sys

sys.path.insert(0, "/opt/trn_rl_repo")

import numpy as np
import ml_dtypes

BF16 = ml_dtypes.bfloat16

B, S, E = 4, 4096, 512
H = 8
DH = 64
HPG = 4  # heads per group
GQ = 256  # features per group for each of q/k/v (HPG*DH)
QE = 512  # query extent per attention sweep
NQQ = S // QE  # 8
NTQ = 4  # token chunks for projection phase
TQ = S // NTQ  # 1024
VW = HPG * 65  # 260: per-key-block V width incl. ones columns
NEG = -1.0e10
SCALE = 0.125  # 1/sqrt(DH)

_CACHE = {}


def _build_nc():
    import concourse.bass as bass
    import concourse.tile as tile
    import concourse.mybir as mybir
    from concourse import bacc

    f32 = mybir.dt.float32
    bf16 = mybir.dt.bfloat16
    AF = mybir.ActivationFunctionType
    ALU = mybir.AluOpType

    nc = bacc.Bacc("TRN2", target_bir_lowering=False, debug=False)

    xT = nc.dram_tensor("xT", [E, S], bf16, kind="ExternalInput").ap()
    wqk = nc.dram_tensor("wqk", [E, 512], bf16, kind="ExternalInput").ap()
    bqk = nc.dram_tensor("bqk", [128, 4], f32, kind="ExternalInput").ap()
    wv = nc.dram_tensor("wv", [E, GQ], bf16, kind="ExternalInput").ap()
    bv = nc.dram_tensor("bv", [1, GQ], bf16, kind="ExternalInput").ap()
    wo = nc.dram_tensor("wo", [128, 2 * 512], bf16, kind="ExternalInput").ap()
    out = nc.dram_tensor("out", [S, E], f32, kind="ExternalOutput").ap()

    with tile.TileContext(nc) as tc:
        with (
            tc.tile_pool(name="consts", bufs=1) as cpool,
            tc.tile_pool(name="xt", bufs=4) as xtpool,
            tc.tile_pool(name="qkv", bufs=1) as qkvpool,
            tc.tile_pool(name="pt", bufs=3) as ptpool,
            tc.tile_pool(name="att", bufs=1) as attpool,
            tc.tile_pool(name="eps", bufs=2) as epool,
            tc.tile_pool(name="outs", bufs=2) as opool,
            # PSUM: st pair-tiles double-buffered = 4 banks, ov_e+ov_o = 2,
            # filler (proj/wo) = 2.
            tc.tile_pool(name="st", bufs=1, space="PSUM") as stpool,
            tc.tile_pool(name="ov", bufs=1, space="PSUM") as ovpool,
            tc.tile_pool(name="fl", bufs=1, space="PSUM") as flpool,
        ):
            # ---- constants ----
            wqk_sb = cpool.tile([128, 4 * 512], bf16, name="wqk_sb")
            for ec in range(4):
                nc.sync.dma_start(
                    wqk_sb[:, ec * 512 : (ec + 1) * 512],
                    wqk[ec * 128 : (ec + 1) * 128, :],
                )
            wv_sb = cpool.tile([128, 4 * GQ], bf16, name="wv_sb")
            for ec in range(4):
                nc.sync.dma_start(
                    wv_sb[:, ec * GQ : (ec + 1) * GQ],
                    wv[ec * 128 : (ec + 1) * 128, :],
                )
            wo_sb = cpool.tile([128, 2 * 512], bf16, name="wo_sb")
            nc.sync.dma_start(wo_sb[:], wo[:])
            bqk_sb = cpool.tile([128, 4], f32, name="bqk_sb")
            nc.sync.dma_start(bqk_sb[:], bqk[:])
            bv_sb = cpool.tile([1, GQ], bf16, name="bv_sb")
            nc.sync.dma_start(bv_sb[:], bv[:])
            onesf = cpool.tile([128, 128], f32, name="onesf")
            nc.vector.memset(onesf[:], 1.0)
            ones_row = cpool.tile([1, 128], bf16, name="ones_row")
            nc.vector.tensor_copy(ones_row[:], onesf[0:1, :])
            maskf = cpool.tile([128, 128], f32, name="maskf")
            nc.vector.memset(maskf[:], 0.0)
            nc.gpsimd.affine_select(
                out=maskf[:], in_=maskf[:], compare_op=ALU.is_ge, fill=NEG,
                base=0, pattern=[[1, 128]], channel_multiplier=-1,
            )
            maskT = cpool.tile([128, 128], bf16, name="maskT")
            nc.vector.tensor_copy(maskT[:], maskf[:])
            identf = cpool.tile([128, 128], f32, name="identf")
            nc.vector.memset(identf[:], 0.0)
            nc.gpsimd.affine_select(
                out=identf[:], in_=identf[:], compare_op=ALU.not_equal, fill=1.0,
                base=0, pattern=[[-1, 128]], channel_multiplier=1,
            )
            ident = cpool.tile([128, 128], bf16, name="ident")
            nc.vector.tensor_copy(ident[:], identf[:])

            # persistent qT/kT tiles: [pair A/B][tq] each [128, 1024]
            # pair A rows 0:64 = head0 dh, 64:128 = head1; pair B = heads 2,3
            qt = [
                [qkvpool.tile([128, TQ], bf16, name=f"qt{ab}_{t}") for t in range(NTQ)]
                for ab in range(2)
            ]
            kt = [
                [qkvpool.tile([128, TQ], bf16, name=f"kt{ab}_{t}") for t in range(NTQ)]
                for ab in range(2)
            ]
            vt = [
                qkvpool.tile([128, 8 * VW], bf16, name=f"vt_{t}") for t in range(NTQ)
            ]
            # attention outputs, per 512-query sweep: pair tiles [128, 512]
            # (rows 0:64 head even dh, 64:128 head odd) for K=128 Wo matmuls
            attt = [
                [attpool.tile([128, QE], bf16, name=f"at{q}_{p}") for p in range(2)]
                for q in range(NQQ)
            ]

            # ---- projection / Wo emitted as "filler" groups ----
            xts_cur = [None]

            def g_xload(tq):
                xts = []
                for ec in range(4):
                    xtile = xtpool.tile([128, TQ], bf16, name="xtile", tag="xtile")
                    nc.sync.dma_start(
                        xtile[:],
                        xT[ec * 128 : (ec + 1) * 128, tq * TQ : (tq + 1) * TQ],
                    )
                    xts.append(xtile)
                xts_cur[0] = xts
                v_tile = vt[tq]
                nc.vector.tensor_copy(
                    v_tile.rearrange("p (t h d) -> p t h d", t=8, h=HPG)[:, :, :, 64:65],
                    onesf[:, 0:32].rearrange("p (t h d) -> p t h d", t=8, h=HPG),
                )

            def g_qk(tq, fc, th):
                xts = xts_cur[0]
                dest = (qt if fc < 2 else kt)[fc % 2][tq]
                ps = flpool.tile([128, 512], f32, name="fps", tag=("fl_e", "fl_o")[th])
                for ec in range(4):
                    nc.tensor.matmul(
                        ps[:],
                        lhsT=wqk_sb[:, ec * 512 + fc * 128 : ec * 512 + (fc + 1) * 128],
                        rhs=xts[ec][:, th * 512 : (th + 1) * 512],
                        start=(ec == 0),
                        stop=(ec == 3),
                    )
                nc.vector.tensor_scalar_add(
                    dest[:, th * 512 : (th + 1) * 512], ps[:], bqk_sb[:, fc : fc + 1]
                )

            def g_v(tq, tb):
                xts = xts_cur[0]
                v_tile = vt[tq]
                vps = flpool.tile(
                    [128, GQ], f32, name="fvps", tag=("fl_e", "fl_o")[tb % 2]
                )
                for ec in range(4):
                    nc.tensor.matmul(
                        vps[:],
                        lhsT=xts[ec][:, tb * 128 : (tb + 1) * 128],
                        rhs=wv_sb[:, ec * GQ : (ec + 1) * GQ],
                        start=(ec == 0),
                        stop=False,
                    )
                nc.tensor.matmul(
                    vps[:], lhsT=ones_row[:], rhs=bv_sb[:], start=False, stop=True
                )
                nc.vector.tensor_copy(
                    v_tile[:, tb * VW : (tb + 1) * VW].rearrange(
                        "p (h d) -> p h d", h=HPG
                    )[:, :, 0:64],
                    vps.rearrange("p (h d) -> p h d", h=HPG),
                )

            osb_cur = [None]

            def g_wo(qq, tb):
                # out rows [qq*1024 + tb*128 ...]: contract both pair tiles
                if tb == 0 or tb == 4:
                    osb_cur[0] = opool.tile(
                        [128, 4 * 512], f32, name="osb", tag=f"osb{qq % 2}"
                    )
                out_sb = osb_cur[0]
                qqp, tbl = (2 * qq + tb // 4), tb % 4
                wops = flpool.tile(
                    [128, 512], f32, name="fwops", tag=("fl_e", "fl_o")[tb % 2]
                )
                for p in range(2):
                    nc.tensor.matmul(
                        wops[:],
                        lhsT=attt[qqp][p][:, tbl * 128 : (tbl + 1) * 128],
                        rhs=wo_sb[:, p * 512 : (p + 1) * 512],
                        start=(p == 0),
                        stop=(p == 1),
                    )
                nc.vector.tensor_copy(out_sb[:, tbl * 512 : (tbl + 1) * 512], wops[:])
                if tbl == 3:
                    half = tb // 4
                    nc.sync.dma_start(
                        out[
                            qq * 1024 + half * 512 : qq * 1024 + (half + 1) * 512, :
                        ].rearrange("(t p) c -> p t c", p=128),
                        out_sb.rearrange("p (t c) -> p t c", t=4),
                    )

            # filler queue: (min_sweep, emit_fn); force-drained by barrier()
            fillq = []

            def p1_groups(tq, min_sweep):
                fillq.append((min_sweep, lambda tq=tq: g_xload(tq)))
                for gi, fc in enumerate((0, 2, 1, 3)):
                    for th in range(2):
                        fillq.append(
                            (min_sweep, lambda tq=tq, fc=fc, th=th: g_qk(tq, fc, th))
                        )
                for tb in range(8):
                    fillq.append((min_sweep, lambda tq=tq, tb=tb: g_v(tq, tb)))

            def wo_groups(qq, min_sweep):
                for tb in range(8):
                    fillq.append((min_sweep, lambda qq=qq, tb=tb: g_wo(qq, tb)))

            def pull_filler(sweep):
                if fillq and fillq[0][0] <= sweep:
                    fillq.pop(0)[1]()

            def drain_filler(upto_min_sweep):
                while fillq and fillq[0][0] < upto_min_sweep:
                    fillq.pop(0)[1]()

            def epilogue(qqp, pr, half, ovt):
                # ovt rows 0:64 = head output [dh, QE], row 64 = softmax sums
                denrow = epool.tile([1, QE], f32, name="denrow", tag="denrow")
                nc.vector.tensor_copy(denrow[:], ovt[64:65, :])
                sbc = epool.tile([DH, QE], f32, name="sbc", tag="sbc")
                nc.sync.dma_start(sbc[:], denrow.unsqueeze(1).to_broadcast([1, DH, QE]))
                rbc = epool.tile([DH, QE], f32, name="rbc", tag="rbc")
                scr = epool.tile([DH, QE], f32, name="scr", tag="rscr", bufs=1)
                nc.vector.reciprocal_approx_accurate(out=rbc[:], in_=sbc[:], scratch=scr[:])
                nc.vector.tensor_tensor(
                    attt[qqp][pr][half * DH : (half + 1) * DH, :],
                    ovt[0:DH, :],
                    rbc[:],
                    ALU.mult,
                )

            # ---- software-pipelined attention ----
            stepc = [0]  # global st-buffer parity
            pend_pv = [None]
            pend_epi = [None]

            def flush_slot(sweep):
                had_pv = pend_pv[0] is not None
                if had_pv:
                    pend_pv[0]()
                    pend_pv[0] = None
                if pend_epi[0] is not None:
                    pend_epi[0]()
                    pend_epi[0] = None
                if had_pv:
                    pull_filler(sweep)

            def att_unit(qq, pr):
                nkb = 4 * qq + 4
                ov_e = ovpool.tile([65, QE], f32, name="ov_e", tag="ov_e")
                ov_o = ovpool.tile([65, QE], f32, name="ov_o", tag="ov_o")
                qtile = qt[pr][qq // 2]
                qoff = (qq % 2) * 512
                for kb in range(nkb):
                    tqk, kbl = kb // 8, kb % 8
                    qs = max(0, (kb - 4 * qq) * 128)
                    st = stpool.tile(
                        [128, 2 * QE], f32, name="st", tag=f"st{stepc[0] % 2}"
                    )
                    stepc[0] += 1
                    for hh in range(2):
                        nc.tensor.matmul(
                            st[:, hh * 512 + qs : hh * 512 + 512],
                            lhsT=kt[pr][tqk][hh * 64 : hh * 64 + 64, kbl * 128 : (kbl + 1) * 128],
                            rhs=qtile[hh * 64 : hh * 64 + 64, qoff + qs : qoff + 512],
                            start=True,
                            stop=True,
                        )
                    if kb >= 4 * qq:  # diagonal: accumulate causal mask on PE
                        for hh in range(2):
                            nc.tensor.matmul(
                                st[:, hh * 512 + qs : hh * 512 + qs + 128],
                                lhsT=ident[:],
                                rhs=maskT[:],
                                start=False,
                                stop=True,
                                skip_group_check=True,
                            )
                    pt = ptpool.tile([128, 2 * QE], bf16, name="pt", tag="pt")
                    if qs == 0:
                        nc.scalar.activation(
                            pt[:, 0 : 2 * QE], st[:, 0 : 2 * QE], AF.Exp,
                            bias=0.0, scale=SCALE,
                        )
                    else:
                        for hh in range(2):
                            nc.scalar.activation(
                                pt[:, hh * 512 + qs : hh * 512 + 512],
                                st[:, hh * 512 + qs : hh * 512 + 512],
                                AF.Exp, bias=0.0, scale=SCALE,
                            )
                    flush_slot(qq)

                    def pv(tqk=tqk, kbl=kbl, qs=qs, kb=kb, pt=pt, pr=pr,
                           ov_e=ov_e, ov_o=ov_o, nkb=nkb):
                        nc.tensor.matmul(
                            ov_e[:, qs:QE],
                            lhsT=vt[tqk][:, kbl * VW + 2 * pr * 65 : kbl * VW + (2 * pr + 1) * 65],
                            rhs=pt[:, qs:512],
                            start=(kb == 0),
                            stop=(kb == nkb - 1),
                            skip_group_check=True,
                        )
                        nc.tensor.matmul(
                            ov_o[:, qs:QE],
                            lhsT=vt[tqk][:, kbl * VW + (2 * pr + 1) * 65 : kbl * VW + (2 * pr + 2) * 65],
                            rhs=pt[:, 512 + qs : 1024],
                            start=(kb == 0),
                            stop=(kb == nkb - 1),
                            skip_group_check=True,
                        )

                    pend_pv[0] = pv

                def epi(qq=qq, pr=pr, ov_e=ov_e, ov_o=ov_o):
                    epilogue(qq, pr, 0, ov_e)
                    epilogue(qq, pr, 1, ov_o)

                pend_epi[0] = epi

            # ---- schedule ----
            # p1(0) up front; p1(1..3) + wo(0..2) interleaved as filler;
            # wo(3) at the end.
            p1_groups(0, min_sweep=-1)
            drain_filler(0)
            for tq in range(1, NTQ):
                p1_groups(tq, min_sweep=2 * (tq - 1))
            wo_groups(0, min_sweep=2)
            wo_groups(1, min_sweep=4)
            wo_groups(2, min_sweep=6)

            # sort filler by min_sweep preserving emission order per class
            fillq.sort(key=lambda e: e[0])

            for qq in range(NQQ):
                drain_filler(2 * ((qq + 1) // 2))  # p1(tq) done before sweep 2tq
                for pr in range(2):
                    att_unit(qq, pr)
            flush_slot(NQQ)
            drain_filler(10**9)
            wo_groups(3, min_sweep=0)
            drain_filler(10**9)

    nc.finalize()
    return nc


def _get_nc():
    if "nc" not in _CACHE:
        _CACHE["nc"] = _build_nc()
    return _CACHE["nc"]


def _make_in_maps(x, Wqkv, bqkv, Wo):
    in_maps = []
    for core in range(8):
        b, g = core // 2, core % 2
        qs, ks, vs = g * GQ, 512 + g * GQ, 1024 + g * GQ
        wqk_np = np.ascontiguousarray(
            np.concatenate([Wqkv[:, qs : qs + GQ], Wqkv[:, ks : ks + GQ]], axis=1)
        ).astype(BF16)
        bqk_np = np.ascontiguousarray(
            np.concatenate([bqkv[qs : qs + GQ], bqkv[ks : ks + GQ]]).reshape(4, 128).T
        )
        wv_np = np.ascontiguousarray(Wqkv[:, vs : vs + GQ]).astype(BF16)
        bv_np = np.ascontiguousarray(bqkv[vs : vs + GQ].reshape(1, GQ)).astype(BF16)
        wo_g = Wo[g * GQ : (g + 1) * GQ, :]
        # pair p block rows = heads 2p,2p+1 stacked = wo_g[p*128:(p+1)*128]
        wo_np = np.ascontiguousarray(
            np.concatenate([wo_g[0:128, :], wo_g[128:256, :]], axis=1)
        ).astype(BF16)
        in_maps.append(
            {
                "xT": np.ascontiguousarray(x[b].T).astype(BF16),
                "wqk": wqk_np,
                "bqk": bqk_np,
                "wv": wv_np,
                "bv": bv_np,
                "wo": wo_np,
            }
        )
    return in_maps


def kernel(x, Wqkv, bqkv, Wo, bo, **run_kwargs):
    from concourse.bass_utils import run_bass_kernel_spmd

    x = np.asarray(x, dtype=np.float32)
    Wqkv = np.asarray(Wqkv, dtype=np.float32)
    bqkv = np.asarray(bqkv, dtype=np.float32)
    Wo = np.asarray(Wo, dtype=np.float32)
    bo = np.asarray(bo, dtype=np.float32)

    nc = _get_nc()
    in_maps = _make_in_maps(x, Wqkv, bqkv, Wo)

    res = run_bass_kernel_spmd(nc, in_maps, core_ids=list(range(8)), **run_kwargs)
    _CACHE["last_results"] = res

    out = np.empty((B, S, E), dtype=np.float32)
    for b in range(B):
        out[b] = res.results[2 * b]["out"] + res.results[2 * b + 1]["out"] + bo
    return out


# revision 10
# speedup vs baseline: 1.7484x; 1.1906x over previous
"""Multi-head causal attention (B=4, S=4096, E=512, H=8) on 8 trn2 NeuronCores.

Sharding: core = (batch b, head-group g of 4 heads); 4 batches x 2 groups = 8 cores.
Each core computes qkv projection for its group's heads, causal attention, and a
partial output projection (its heads' rows of Wo). Host sums the two partials per
batch and adds bo.

All matmul operands are bf16 (fp32/f32r runs the PE in multi-pass mode; bf16
streams 1 col/cycle). PSUM accumulation stays fp32.

The attention loop is software-pipelined to keep the PE continuously busy (the
PE clock drops to half speed after any idle gap, which was the dominant cost):
  - 512-query sweeps; S^T for both heads of a pair lives in ONE [128, 1024]
    PSUM tile (head e cols 0:512, head o 512:1024), double-buffered (4 banks),
    so QK(kb+1) never waits for exp(kb).
  - PV runs one kb behind QK; exp(kb) overlaps QK(kb+1)+PV(kb-1).
  - ov accumulators [65, 512] x2 (2 banks); remaining 2 banks feed interleaved
    projection/Wo "filler" matmuls pulled one group per kb slot.
  - V is stored token-major with a ones column per (kb, head) so the PV matmul
    also produces softmax denominators (row 64 of ov).
"""

import sys

sys.path.insert(0, "/opt/trn_rl_repo")

import numpy as np
import ml_dtypes

BF16 = ml_dtypes.bfloat16

B, S, E = 4, 4096, 512
H = 8
DH = 64
HPG = 4  # heads per group
GQ = 256  # features per group for each of q/k/v (HPG*DH)
QE = 512  # query extent per attention sweep
NQQ = S // QE  # 8
NTQ = 4  # token chunks for projection phase
TQ = S // NTQ  # 1024
VW = HPG * 65  # 260: per-key-block V width incl. ones columns
NEG = -1.0e10
SCALE = 0.125  # 1/sqrt(DH)

_CACHE = {}


def _build_nc():
    import concourse.bass as bass
    import concourse.tile as tile
    import concourse.mybir as mybir
    from concourse import bacc

    f32 = mybir.dt.float32
    bf16 = mybir.dt.bfloat16
    AF = mybir.ActivationFunctionType
    ALU = mybir.AluOpType

    nc = bacc.Bacc("TRN2", target_bir_lowering=False, debug=False)

    xT = nc.dram_tensor("xT", [E, S], bf16, kind="ExternalInput").ap()
    wqk = nc.dram_tensor("wqk", [E, 512], bf16, kind="ExternalInput").ap()
    bqk = nc.dram_tensor("bqk", [128, 4], f32, kind="ExternalInput").ap()
    wv = nc.dram_tensor("wv", [E, GQ], bf16, kind="ExternalInput").ap()
    bv = nc.dram_tensor("bv", [1, GQ], bf16, kind="ExternalInput").ap()
    wo = nc.dram_tensor("wo", [128, 2 * 512], bf16, kind="ExternalInput").ap()
    out = nc.dram_tensor("out", [S, E], f32, kind="ExternalOutput").ap()

    with tile.TileContext(nc) as tc:
        with (
            tc.tile_pool(name="consts", bufs=1) as cpool,
            tc.tile_pool(name="xt", bufs=4) as xtpool,
            tc.tile_pool(name="qkv", bufs=1) as qkvpool,
            tc.tile_pool(name="pt", bufs=3) as ptpool,
            tc.tile_pool(name="att", bufs=1) as attpool,
            tc.tile_pool(name="eps", bufs=2) as epool,
            tc.tile_pool(name="outs", bufs=2) as opool,
            # PSUM: st pair-tiles double-buffered = 4 banks; ov_e+ov_o
            # double-buffered by unit parity = 4 banks. Filler (proj/wo)
            # psum borrows the idle-parity ov banks.
            tc.tile_pool(name="st", bufs=1, space="PSUM") as stpool,
            tc.tile_pool(name="ov", bufs=1, space="PSUM") as ovpool,
        ):
            # ---- constants ----
            wqk_sb = cpool.tile([128, 4 * 512], bf16, name="wqk_sb")
            for ec in range(4):
                nc.sync.dma_start(
                    wqk_sb[:, ec * 512 : (ec + 1) * 512],
                    wqk[ec * 128 : (ec + 1) * 128, :],
                )
            wv_sb = cpool.tile([128, 4 * GQ], bf16, name="wv_sb")
            for ec in range(4):
                nc.sync.dma_start(
                    wv_sb[:, ec * GQ : (ec + 1) * GQ],
                    wv[ec * 128 : (ec + 1) * 128, :],
                )
            wo_sb = cpool.tile([128, 2 * 512], bf16, name="wo_sb")
            nc.sync.dma_start(wo_sb[:], wo[:])
            bqk_sb = cpool.tile([128, 4], f32, name="bqk_sb")
            nc.sync.dma_start(bqk_sb[:], bqk[:])
            bv_sb = cpool.tile([1, GQ], bf16, name="bv_sb")
            nc.sync.dma_start(bv_sb[:], bv[:])
            onesf = cpool.tile([128, 128], f32, name="onesf")
            nc.vector.memset(onesf[:], 1.0)
            ones_row = cpool.tile([1, 128], bf16, name="ones_row")
            nc.vector.tensor_copy(ones_row[:], onesf[0:1, :])
            maskf = cpool.tile([128, 128], f32, name="maskf")
            nc.vector.memset(maskf[:], 0.0)
            nc.gpsimd.affine_select(
                out=maskf[:], in_=maskf[:], compare_op=ALU.is_ge, fill=NEG,
                base=0, pattern=[[1, 128]], channel_multiplier=-1,
            )
            maskT = cpool.tile([128, 128], bf16, name="maskT")
            nc.vector.tensor_copy(maskT[:], maskf[:])
            identf = cpool.tile([128, 128], f32, name="identf")
            nc.vector.memset(identf[:], 0.0)
            nc.gpsimd.affine_select(
                out=identf[:], in_=identf[:], compare_op=ALU.not_equal, fill=1.0,
                base=0, pattern=[[-1, 128]], channel_multiplier=1,
            )
            ident = cpool.tile([128, 128], bf16, name="ident")
            nc.vector.tensor_copy(ident[:], identf[:])

            # persistent qT/kT tiles: [pair A/B][tq] each [128, 1024]
            # pair A rows 0:64 = head0 dh, 64:128 = head1; pair B = heads 2,3
            qt = [
                [qkvpool.tile([128, TQ], bf16, name=f"qt{ab}_{t}") for t in range(NTQ)]
                for ab in range(2)
            ]
            kt = [
                [qkvpool.tile([128, TQ], bf16, name=f"kt{ab}_{t}") for t in range(NTQ)]
                for ab in range(2)
            ]
            vt = [
                qkvpool.tile([128, 8 * VW], bf16, name=f"vt_{t}") for t in range(NTQ)
            ]
            # attention outputs, per 512-query sweep: pair tiles [128, 512]
            # (rows 0:64 head even dh, 64:128 head odd) for K=128 Wo matmuls
            attt = [
                [attpool.tile([128, QE], bf16, name=f"at{q}_{p}") for p in range(2)]
                for q in range(NQQ)
            ]

            # ---- projection / Wo emitted as "filler" groups ----
            # filler psum borrows the ov banks of the currently-idle parity
            borrow = [("ove1", "ovo1")]
            xts_cur = [None]

            def g_xload(tq):
                xts = []
                for ec in range(4):
                    xtile = xtpool.tile([128, TQ], bf16, name="xtile", tag="xtile")
                    nc.sync.dma_start(
                        xtile[:],
                        xT[ec * 128 : (ec + 1) * 128, tq * TQ : (tq + 1) * TQ],
                    )
                    xts.append(xtile)
                xts_cur[0] = xts
                v_tile = vt[tq]
                nc.vector.tensor_copy(
                    v_tile.rearrange("p (t h d) -> p t h d", t=8, h=HPG)[:, :, :, 64:65],
                    onesf[:, 0:32].rearrange("p (t h d) -> p t h d", t=8, h=HPG),
                )

            def g_qk(tq, fc, th):
                xts = xts_cur[0]
                dest = (qt if fc < 2 else kt)[fc % 2][tq]
                ps = ovpool.tile([128, 512], f32, name="fps", tag=borrow[0][th])
                for ec in range(4):
                    nc.tensor.matmul(
                        ps[:],
                        lhsT=wqk_sb[:, ec * 512 + fc * 128 : ec * 512 + (fc + 1) * 128],
                        rhs=xts[ec][:, th * 512 : (th + 1) * 512],
                        start=(ec == 0),
                        stop=(ec == 3),
                    )
                nc.vector.tensor_scalar_add(
                    dest[:, th * 512 : (th + 1) * 512], ps[:], bqk_sb[:, fc : fc + 1]
                )

            def g_v(tq, tb):
                xts = xts_cur[0]
                v_tile = vt[tq]
                vps = ovpool.tile(
                    [128, GQ], f32, name="fvps", tag=borrow[0][tb % 2]
                )
                for ec in range(4):
                    nc.tensor.matmul(
                        vps[:],
                        lhsT=xts[ec][:, tb * 128 : (tb + 1) * 128],
                        rhs=wv_sb[:, ec * GQ : (ec + 1) * GQ],
                        start=(ec == 0),
                        stop=False,
                    )
                nc.tensor.matmul(
                    vps[:], lhsT=ones_row[:], rhs=bv_sb[:], start=False, stop=True
                )
                nc.vector.tensor_copy(
                    v_tile[:, tb * VW : (tb + 1) * VW].rearrange(
                        "p (h d) -> p h d", h=HPG
                    )[:, :, 0:64],
                    vps.rearrange("p (h d) -> p h d", h=HPG),
                )

            osb_cur = [None]

            def g_wo(qq, tb):
                # out rows [qq*1024 + tb*128 ...]: contract both pair tiles
                if tb == 0 or tb == 4:
                    osb_cur[0] = opool.tile(
                        [128, 4 * 512], f32, name="osb", tag=f"osb{qq % 2}"
                    )
                out_sb = osb_cur[0]
                qqp, tbl = (2 * qq + tb // 4), tb % 4
                wops = ovpool.tile(
                    [128, 512], f32, name="fwops", tag=borrow[0][tb % 2]
                )
                for p in range(2):
                    nc.tensor.matmul(
                        wops[:],
                        lhsT=attt[qqp][p][:, tbl * 128 : (tbl + 1) * 128],
                        rhs=wo_sb[:, p * 512 : (p + 1) * 512],
                        start=(p == 0),
                        stop=(p == 1),
                    )
                nc.vector.tensor_copy(out_sb[:, tbl * 512 : (tbl + 1) * 512], wops[:])
                if tbl == 3:
                    half = tb // 4
                    nc.sync.dma_start(
                        out[
                            qq * 1024 + half * 512 : qq * 1024 + (half + 1) * 512, :
                        ].rearrange("(t p) c -> p t c", p=128),
                        out_sb.rearrange("p (t c) -> p t c", t=4),
                    )

            # filler queue: (min_sweep, emit_fn); force-drained by barrier()
            fillq = []

            def p1_groups(tq, min_sweep):
                fillq.append((min_sweep, lambda tq=tq: g_xload(tq)))
                for gi, fc in enumerate((0, 2, 1, 3)):
                    for th in range(2):
                        fillq.append(
                            (min_sweep, lambda tq=tq, fc=fc, th=th: g_qk(tq, fc, th))
                        )
                for tb in range(8):
                    fillq.append((min_sweep, lambda tq=tq, tb=tb: g_v(tq, tb)))

            def wo_groups(qq, min_sweep):
                for tb in range(8):
                    fillq.append((min_sweep, lambda qq=qq, tb=tb: g_wo(qq, tb)))

            def pull_filler(sweep):
                if fillq and fillq[0][0] <= sweep:
                    fillq.pop(0)[1]()

            def drain_filler(upto_min_sweep):
                while fillq and fillq[0][0] < upto_min_sweep:
                    fillq.pop(0)[1]()

            def epilogue(qqp, pr, half, ovt):
                # ovt rows 0:64 = head output [dh, QE], row 64 = softmax sums
                denrow = epool.tile([1, QE], f32, name="denrow", tag="denrow")
                nc.vector.tensor_copy(denrow[:], ovt[64:65, :])
                sbc = epool.tile([DH, QE], f32, name="sbc", tag="sbc")
                nc.sync.dma_start(sbc[:], denrow.unsqueeze(1).to_broadcast([1, DH, QE]))
                rbc = epool.tile([DH, QE], f32, name="rbc", tag="rbc")
                scr = epool.tile([DH, QE], f32, name="scr", tag="rscr", bufs=1)
                nc.vector.reciprocal_approx_accurate(out=rbc[:], in_=sbc[:], scratch=scr[:])
                nc.vector.tensor_tensor(
                    attt[qqp][pr][half * DH : (half + 1) * DH, :],
                    ovt[0:DH, :],
                    rbc[:],
                    ALU.mult,
                )

            # ---- software-pipelined attention ----
            stepc = [0]  # global st-buffer parity
            pend_pv = [None]
            pend_epi = [None]

            def flush_slot(sweep, fill=True):
                had_pv = pend_pv[0] is not None
                if had_pv:
                    pend_pv[0]()
                    pend_pv[0] = None
                if pend_epi[0] is not None:
                    pend_epi[0]()
                    pend_epi[0] = None
                if had_pv and fill:
                    pull_filler(sweep)

            unitc = [0]  # unit counter: ov parity; filler borrows other parity

            def att_unit(qq, pr):
                nkb = 4 * qq + 4
                par = unitc[0] % 2
                unitc[0] += 1
                borrow[0] = (f"ove{1 - par}", f"ovo{1 - par}")
                ov_e = ovpool.tile([65, QE], f32, name="ov_e", tag=f"ove{par}")
                ov_o = ovpool.tile([65, QE], f32, name="ov_o", tag=f"ovo{par}")
                qtile = qt[pr][qq // 2]
                qoff = (qq % 2) * 512
                for kb in range(nkb):
                    tqk, kbl = kb // 8, kb % 8
                    qs = max(0, (kb - 4 * qq) * 128)
                    st = stpool.tile(
                        [128, 2 * QE], f32, name="st", tag=f"st{stepc[0] % 2}"
                    )
                    stepc[0] += 1
                    for hh in range(2):
                        nc.tensor.matmul(
                            st[:, hh * 512 + qs : hh * 512 + 512],
                            lhsT=kt[pr][tqk][hh * 64 : hh * 64 + 64, kbl * 128 : (kbl + 1) * 128],
                            rhs=qtile[hh * 64 : hh * 64 + 64, qoff + qs : qoff + 512],
                            start=True,
                            stop=True,
                        )
                    if kb >= 4 * qq:  # diagonal: accumulate causal mask on PE
                        for hh in range(2):
                            nc.tensor.matmul(
                                st[:, hh * 512 + qs : hh * 512 + qs + 128],
                                lhsT=ident[:],
                                rhs=maskT[:],
                                start=False,
                                stop=True,
                                skip_group_check=True,
                            )
                    pt = ptpool.tile([128, 2 * QE], bf16, name="pt", tag="pt")
                    if qs == 0:
                        nc.scalar.activation(
                            pt[:, 0 : 2 * QE], st[:, 0 : 2 * QE], AF.Exp,
                            bias=0.0, scale=SCALE,
                        )
                    else:
                        for hh in range(2):
                            nc.scalar.activation(
                                pt[:, hh * 512 + qs : hh * 512 + 512],
                                st[:, hh * 512 + qs : hh * 512 + 512],
                                AF.Exp, bias=0.0, scale=SCALE,
                            )
                    flush_slot(qq, fill=(kb % 2 == 1 and kb >= 3))

                    def pv(tqk=tqk, kbl=kbl, qs=qs, kb=kb, pt=pt, pr=pr,
                           ov_e=ov_e, ov_o=ov_o, nkb=nkb):
                        nc.tensor.matmul(
                            ov_e[:, qs:QE],
                            lhsT=vt[tqk][:, kbl * VW + 2 * pr * 65 : kbl * VW + (2 * pr + 1) * 65],
                            rhs=pt[:, qs:512],
                            start=(kb == 0),
                            stop=(kb == nkb - 1),
                            skip_group_check=True,
                        )
                        nc.tensor.matmul(
                            ov_o[:, qs:QE],
                            lhsT=vt[tqk][:, kbl * VW + (2 * pr + 1) * 65 : kbl * VW + (2 * pr + 2) * 65],
                            rhs=pt[:, 512 + qs : 1024],
                            start=(kb == 0),
                            stop=(kb == nkb - 1),
                            skip_group_check=True,
                        )

                    pend_pv[0] = pv

                def epi(qq=qq, pr=pr, ov_e=ov_e, ov_o=ov_o):
                    epilogue(qq, pr, 0, ov_e)
                    epilogue(qq, pr, 1, ov_o)

                pend_epi[0] = epi

            # ---- schedule ----
            # p1(0) up front; p1(1..3) + wo(0..2) interleaved as filler;
            # wo(3) at the end.
            p1_groups(0, min_sweep=-1)
            drain_filler(0)
            for tq in range(1, NTQ):
                p1_groups(tq, min_sweep=2 * (tq - 1))
            wo_groups(0, min_sweep=2)
            wo_groups(1, min_sweep=4)
            wo_groups(2, min_sweep=6)

            # sort filler by min_sweep preserving emission order per class
            fillq.sort(key=lambda e: e[0])

            for qq in range(NQQ):
                drain_filler(2 * ((qq + 1) // 2))  # p1(tq) done before sweep 2tq
                for pr in range(2):
                    att_unit(qq, pr)
            flush_slot(NQQ)
            drain_filler(10**9)
            wo_groups(3, min_sweep=0)
            drain_filler(10**9)

    nc.finalize()
    return nc


def _get_nc():
    if "nc" not in _CACHE:
        _CACHE["nc"] = _build_nc()
    return _CACHE["nc"]


def _make_in_maps(x, Wqkv, bqkv, Wo):
    in_maps = []
    for core in range(8):
        b, g = core // 2, core % 2
        qs, ks, vs = g * GQ, 512 + g * GQ, 1024 + g * GQ
        wqk_np = np.ascontiguousarray(
            np.concatenate([Wqkv[:, qs : qs + GQ], Wqkv[:, ks : ks + GQ]], axis=1)
        ).astype(BF16)
        bqk_np = np.ascontiguousarray(
            np.concatenate([bqkv[qs : qs + GQ], bqkv[ks : ks + GQ]]).reshape(4, 128).T
        )
        wv_np = np.ascontiguousarray(Wqkv[:, vs : vs + GQ]).astype(BF16)
        bv_np = np.ascontiguousarray(bqkv[vs : vs + GQ].reshape(1, GQ)).astype(BF16)
        wo_g = Wo[g * GQ : (g + 1) * GQ, :]
        # pair p block rows = heads 2p,2p+1 stacked = wo_g[p*128:(p+1)*128]
        wo_np = np.ascontiguousarray(
            np.concatenate([wo_g[0:128, :], wo_g[128:256, :]], axis=1)
        ).astype(BF16)
        in_maps.append(
            {
                "xT": np.ascontiguousarray(x[b].T).astype(BF16),
                "wqk": wqk_np,
                "bqk": bqk_np,
                "wv": wv_np,
                "bv": bv_np,
                "wo": wo_np,
            }
        )
    return in_maps


def kernel(x, Wqkv, bqkv, Wo, bo, **run_kwargs):
    from concourse.bass_utils import run_bass_kernel_spmd

    x = np.asarray(x, dtype=np.float32)
    Wqkv = np.asarray(Wqkv, dtype=np.float32)
    bqkv = np.asarray(bqkv, dtype=np.float32)
    Wo = np.asarray(Wo, dtype=np.float32)
    bo = np.asarray(bo, dtype=np.float32)

    nc = _get_nc()
    in_maps = _make_in_maps(x, Wqkv, bqkv, Wo)

    res = run_bass_kernel_spmd(nc, in_maps, core_ids=list(range(8)), **run_kwargs)
    _CACHE["last_results"] = res

    out = np.empty((B, S, E), dtype=np.float32)
    for b in range(B):
        out[b] = res.results[2 * b]["out"] + res.results[2 * b + 1]["out"] + bo
    return out
